# revision 4
# baseline (speedup 1.0000x reference)
"""Multi-head attention kernel for Trainium2, sharded over 8 NeuronCores.

Problem: q,k,v [4, 16, 2048, 64] f32 -> softmax(q@k^T/sqrt(64))@v.
Sharding: batch*heads = 64 (b,h) pairs -> 8 per core (no communication).

Per-core algorithm, per (b,h) pair (S=2048, D=64):
  phase0: cast q,k f32->bf16 via SBUF into DRAM scratch (elementwise; layout
          agnostic), then xbar DMA-transpose to qT/kT [64, 2048] in SBUF
          (duplicated to partitions 64-127 for PE row-packing).
  main:   for each qi-half (1024 cols) x ki-chunk (128 rows):
            S^T chunk = kT_chunk.T @ qT        (two row-packed bf16 matmuls,
                                                K=64 in rows 0-63 / 64-127)
            P^T = exp(S^T / 8)                 (one ScalarE ACTIVATE,
                                                PSUM->SBUF, bf16 out)
            acc += [V|1]^T @ P^T               (bf16 matmuls accumulating in
                                                PSUM; ones column yields the
                                                softmax denominator row)
  final:  PE-transpose acc [65, 128]-blocks -> [128, 65]; out = num * (1/den)
          per partition (VectorE reciprocal + tensor_scalar); DMA to DRAM.

No max-subtraction is needed: scores ~ N(0,1) after the 1/8 scale, so exp is
far from overflow and softmax is algebraically identical to the reference.
"""

import numpy as np

import concourse.bass as bass
import concourse.tile as tile
from concourse import bacc, mybir
from concourse.bass_utils import run_bass_kernel_spmd

B, H, S, D = 4, 16, 2048, 64
NCORES = 8
BH = (B * H) // NCORES  # (b,h) pairs per core = 8

F32 = mybir.dt.float32
BF16 = mybir.dt.bfloat16

KC = S // 128   # ki chunks of 128 rows        = 16
NH = 2          # qi halves                     (1024 each)
HW_ = S // NH   # qi-half width                 = 1024
NB = HW_ // 512  # 512-wide matmul blocks/half  = 2


def build_attention(tc, out_ap, q_ap, k_ap, v_ap, n_bh=BH):
    nc = tc.nc
    ctx_pools = []

    def pool(name, bufs, space="SBUF"):
        p = tc.alloc_tile_pool(name=name, bufs=bufs, space=space)
        ctx_pools.append(p)
        return p

    singles = pool("singles", 1)
    p0 = pool("p0", 2)          # phase-0 staging (f32 in, bf16 out)
    dram = pool("dram", 2, space="DRAM")
    pqt = pool("qt", 2)         # qT/kT bf16 [128, S]
    pv = pool("pv", 2)          # V_aug + v staging
    ppt = pool("pt", 3)         # exp output P^T bf16
    pfin = pool("fin", 2)       # finalize sbuf tiles
    pout = pool("obuf", 3)
    psum_stage = pool("stage", 2, space="PSUM")   # S^T staging [128,1024] = 2 banks
    psum_acc = pool("acc", 1, space="PSUM")       # PV accumulator [65,1024] = 2 banks
    psum_tp = pool("tp", 2, space="PSUM")         # finalize transpose [128,65]

    # 65x65 identity for PE transposes
    from concourse.masks import make_identity
    ident = singles.tile([65, 65], F32)
    make_identity(nc, ident[:])

    for bh in range(n_bh):
        # ---- phase 0: cast q,k to bf16 in DRAM scratch ----
        tposed = {}
        for name, src in (("q", q_ap), ("k", k_ap)):
            stg = p0.tile([128, S * D // 128], F32, tag="p0f32")
            nc.sync.dma_start(
                out=stg[:], in_=src[bh].rearrange("(p f) d -> p (f d)", p=128)
            )
            stg16 = p0.tile([128, S * D // 128], BF16, tag="p0bf16")
            nc.vector.tensor_copy(stg16[:], stg[:])
            scratch = dram.tile([S, D], BF16, tag=f"{name}16")
            nc.sync.dma_start(
                out=scratch.rearrange("(p f) d -> p (f d)", p=128), in_=stg16[:]
            )
            # xbar transpose DRAM -> SBUF, duplicated into both partition halves
            t = pqt.tile([128, S], BF16, tag=f"{name}T")
            nc.sync.dma_start_transpose(out=t[0:64, :], in_=scratch[:])
            nc.sync.dma_start_transpose(out=t[64:128, :], in_=scratch[:])
            tposed[name] = t
        qT, kT = tposed["q"], tposed["k"]

        # ---- V_aug: [128, KC, 66] bf16; col 64 = ones (denominator trick) ----
        vaug = pv.tile([128, KC, 66], BF16, tag="vaug")
        nc.gpsimd.memset(vaug[:], 1.0)
        vstg = pv.tile([128, KC, D], F32, tag="vstg")
        nc.sync.dma_start(
            out=vstg[:], in_=v_ap[bh].rearrange("(n p) d -> p n d", p=128)
        )
        nc.vector.tensor_copy(vaug[:, :, 0:D], vstg[:])

        for h in range(NH):
            acc = psum_acc.tile([65, HW_], F32, tag="acc")
            for j in range(KC):
                stage = psum_stage.tile([128, HW_], F32, tag="stage")
                # S^T[j*128:(j+1)*128, h*1024:(h+1)*1024] row-packed pair
                for half in range(2):
                    lo, hi = 64 * half, 64 * half + 64
                    q0 = h * HW_ + half * 512
                    nc.tensor.matmul(
                        stage[:, half * 512:half * 512 + 512],
                        lhsT=kT[lo:hi, j * 128:(j + 1) * 128],
                        rhs=qT[lo:hi, q0:q0 + 512],
                        start=True,
                        stop=True,
                    )
                # P^T = exp(S^T / 8), one ACT pass, bf16 out
                pt = ppt.tile([128, HW_], BF16, tag="pt")
                nc.scalar.activation(
                    pt[:], stage[:], mybir.ActivationFunctionType.Exp, scale=0.125
                )
                # acc += V_aug^T @ P^T   (num rows 0-63, den row 64)
                for nb in range(NB):
                    nc.tensor.matmul(
                        acc[:, nb * 512:nb * 512 + 512],
                        lhsT=vaug[:, j, 0:65],
                        rhs=pt[:, nb * 512:nb * 512 + 512],
                        start=(j == 0),
                        stop=(j == KC - 1),
                    )

            # ---- finalize half: transpose back, divide by denominator ----
            accS = pfin.tile([65, HW_], F32, tag="accS")
            nc.vector.tensor_copy(accS[:], acc[:])
            for b in range(HW_ // 128):
                tp = psum_tp.tile([128, 65], F32, tag="tp")
                nc.tensor.transpose(
                    tp[:], accS[:, b * 128:(b + 1) * 128], ident[:]
                )
                rcp = pout.tile([128, 1], F32, tag="rcp")
                nc.vector.reciprocal(rcp[:], tp[:, 64:65])
                obuf = pout.tile([128, D], F32, tag="obuf")
                nc.vector.tensor_scalar_mul(obuf[:], tp[:, 0:D], rcp[:])
                r0 = h * HW_ + b * 128
                nc.sync.dma_start(out=out_ap[bh, r0:r0 + 128, :], in_=obuf[:])

    for p in reversed(ctx_pools):
        p.release()


_CACHE = {}


def _get_compiled(n_bh=BH):
    key = ("nc", n_bh)
    if key in _CACHE:
        return _CACHE[key]
    nc = bacc.Bacc("TRN2", target_bir_lowering=False, debug=False)
    q = nc.dram_tensor("q", [n_bh, S, D], F32, kind="ExternalInput").ap()
    k = nc.dram_tensor("k", [n_bh, S, D], F32, kind="ExternalInput").ap()
    v = nc.dram_tensor("v", [n_bh, S, D], F32, kind="ExternalInput").ap()
    out = nc.dram_tensor("out", [n_bh, S, D], F32, kind="ExternalOutput").ap()
    with tile.TileContext(nc) as tc:
        build_attention(tc, out, q, k, v, n_bh=n_bh)
    nc.compile()
    _CACHE[key] = nc
    return nc


def kernel(q, k, v):
    nc = _get_compiled()
    qf = np.ascontiguousarray(np.asarray(q), dtype=np.float32).reshape(B * H, S, D)
    kf = np.ascontiguousarray(np.asarray(k), dtype=np.float32).reshape(B * H, S, D)
    vf = np.ascontiguousarray(np.asarray(v), dtype=np.float32).reshape(B * H, S, D)
    in_maps = [
        {
            "q": qf[i * BH:(i + 1) * BH],
            "k": kf[i * BH:(i + 1) * BH],
            "v": vf[i * BH:(i + 1) * BH],
        }
        for i in range(NCORES)
    ]
    res = run_bass_kernel_spmd(nc, in_maps, list(range(NCORES)))
    outs = np.concatenate([res.results[i]["out"] for i in range(NCORES)], axis=0)
    return outs.reshape(B, H, S, D).astype(np.float32)


# revision 5
# speedup vs baseline: 11.4962x; 11.4962x over previous
"""Multi-head attention kernel for Trainium2, sharded over 8 NeuronCores.

Problem: q,k,v [4, 16, 2048, 64] f32 -> softmax(q@k^T/sqrt(64))@v.
Sharding: batch*heads = 64 (b,h) pairs -> 8 per core (no communication).

Per-core algorithm, per (b,h) pair (S=2048, D=64):
  load:   q,k arrive as [128, 16, 64] f32 tiles; each [128, 64] tile is
          PE-transposed (fp32 transpose mode) to [64, 128] in PSUM and
          copied (with f32->bf16 cast) to SBUF by VectorE:
            qT [128, 2048]: d on partitions, duplicated to partitions 64-127
            kT [128, 8*128]: even ki-chunks on partitions 0-63, odd on 64-127
          v is DMA'd by the gpsimd SWDGE with f32->bf16 cast in flight into
          V_aug [128, 16, 128]: cols 0-63 = v, col 64 = ones (softmax
          denominator trick), cols 65-127 = zero padding (enables FWL).
  main:   for each qi-half (1024) x block b (512) x chunk-pair m (2x128 ki):
            S^T pair = kT_pair.T @ qT  (two row-packed bf16 matmuls, K=64,
                                        running concurrently on PE row groups
                                        0-63 / 64-127)
            P^T = exp(S^T / 8)         (one ScalarE ACTIVATE, FD=1024,
                                        PSUM->SBUF, bf16 out)
            acc[:, b] += V_aug^T @ P^T (bf16 matmuls accumulating in PSUM;
                                        row 64 of acc = denominators)
  final:  PE-transpose acc [65, 128]-blocks back -> [128, 65]; multiply by
          VectorE reciprocal of the denominator column; batch the 8 result
          blocks per half into one [128, 8, 64] tile; single DMA to DRAM.

No max-subtraction is needed: scores ~ N(0,1) after the 1/8 scale, so exp is
far from overflow and softmax is algebraically identical to the reference.

Engine budget per core (estimates): ScalarE exp ~237us (bottleneck),
TensorE ~210us, VectorE ~115us, DMA ~16MB via async HWDGE/SWDGE.
"""

import numpy as np

import concourse.bass as bass
import concourse.tile as tile
from concourse import bacc, mybir
from concourse.bass_utils import run_bass_kernel_spmd

B, H, S, D = 4, 16, 2048, 64
NCORES = 8
BH = (B * H) // NCORES  # (b,h) pairs per core = 8

F32 = mybir.dt.float32
BF16 = mybir.dt.bfloat16

KC = S // 128    # ki chunks of 128 rows       = 16
NH = 2           # qi halves                    (1024 each)
HW_ = S // NH    # qi-half width                = 1024
NB = HW_ // 512  # 512-wide blocks per half     = 2


def build_attention(tc, out_ap, q_ap, k_ap, v_ap, n_bh=BH):
    nc = tc.nc
    pools = []

    def pool(name, bufs, space="SBUF"):
        p = tc.alloc_tile_pool(name=name, bufs=bufs, space=space)
        pools.append(p)
        return p

    singles = pool("singles", 1)
    pin = pool("pin", 2)        # q/k f32 natural tiles
    pqt = pool("pqt", 2)        # qT / kT bf16
    pv = pool("pv", 2)          # V_aug
    ppt = pool("ppt", 3)        # exp output P^T bf16
    pfin = pool("pfin", 2)      # finalize sbuf staging
    psml = pool("psml", 3)      # small finalize tiles
    pob = pool("pob", 2)        # batched output tiles
    psum_stage = pool("stage", 2, space="PSUM")  # S^T staging, 2 banks each
    psum_acc = pool("acc", 1, space="PSUM")      # PV accumulator, 2 banks
    psum_tp = pool("tp", 2, space="PSUM")        # transposes, 1 bank each

    from concourse.masks import make_identity
    ident = singles.tile([128, 128], F32)
    make_identity(nc, ident[:])

    for bh in range(n_bh):
        # ---- load + on-chip transpose of q, k ----
        qstage = pin.tile([128, KC, D], F32, tag="qstage")
        nc.sync.dma_start(
            out=qstage[:], in_=q_ap[bh].rearrange("(n p) d -> p n d", p=128)
        )
        kstage = pin.tile([128, KC, D], F32, tag="kstage")
        nc.sync.dma_start(
            out=kstage[:], in_=k_ap[bh].rearrange("(n p) d -> p n d", p=128)
        )
        qT = pqt.tile([128, S], BF16, tag="qT")
        kT = pqt.tile([128, (KC // 2) * 128], BF16, tag="kT")
        for j in range(KC):
            tpq = psum_tp.tile([64, 128], F32, tag="tp")
            nc.tensor.transpose(tpq[:], qstage[:, j, :], ident[:])
            nc.vector.tensor_copy(qT[0:64, j * 128:(j + 1) * 128], tpq[:])
        # duplicate qT into partitions 64-127 (for PE row-packing)
        nc.vector.tensor_copy(qT[64:128, :], qT[0:64, :])
        for j in range(KC):
            tpk = psum_tp.tile([64, 128], F32, tag="tp")
            nc.tensor.transpose(tpk[:], kstage[:, j, :], ident[:])
            row = 64 * (j % 2)
            m = j // 2
            nc.vector.tensor_copy(
                kT[row:row + 64, m * 128:(m + 1) * 128], tpk[:]
            )

        # ---- V_aug [128, KC, 128]: v | ones | zeros, bf16 ----
        vaug = pv.tile([128, KC, 128], BF16, tag="vaug")
        nc.gpsimd.memset(vaug[:], 0.0)
        nc.gpsimd.memset(vaug[:, :, D:D + 1], 1.0)
        # SWDGE DMA casts f32->bf16 in flight
        nc.gpsimd.dma_start(
            out=vaug[:, :, 0:D], in_=v_ap[bh].rearrange("(n p) d -> p n d", p=128)
        )

        for h in range(NH):
            acc = psum_acc.tile([128, HW_], F32, tag="acc")
            for b in range(NB):
                q0 = h * HW_ + b * 512
                for m in range(KC // 2):
                    # S^T for chunks (2m, 2m+1) x qi block b: row-packed pair
                    stage = psum_stage.tile([128, 2, 512], F32, tag="stage")
                    nc.tensor.matmul(
                        stage[:, 0, :],
                        lhsT=kT[0:64, m * 128:(m + 1) * 128],
                        rhs=qT[0:64, q0:q0 + 512],
                        start=True, stop=True,
                    )
                    nc.tensor.matmul(
                        stage[:, 1, :],
                        lhsT=kT[64:128, m * 128:(m + 1) * 128],
                        rhs=qT[64:128, q0:q0 + 512],
                        start=True, stop=True,
                    )
                    # P^T = exp(S^T/8) for both chunks in one ACTIVATE
                    pt = ppt.tile([128, 2, 512], BF16, tag="pt")
                    nc.scalar.activation(
                        pt[:], stage[:], mybir.ActivationFunctionType.Exp,
                        scale=0.125,
                    )
                    # acc[:, b] += V_aug^T @ P^T for the two chunks
                    nc.tensor.matmul(
                        acc[:, b * 512:(b + 1) * 512],
                        lhsT=vaug[:, 2 * m, :],
                        rhs=pt[:, 0, :],
                        start=(m == 0), stop=False,
                    )
                    nc.tensor.matmul(
                        acc[:, b * 512:(b + 1) * 512],
                        lhsT=vaug[:, 2 * m + 1, :],
                        rhs=pt[:, 1, :],
                        start=False, stop=(m == KC // 2 - 1),
                    )

            # ---- finalize half: transpose back, divide by denominator ----
            accS = pfin.tile([65, HW_], F32, tag="accS")
            nc.vector.tensor_copy(accS[:], acc[0:65, :])
            obuf = pob.tile([128, HW_ // 128, D], F32, tag="obuf")
            for b in range(HW_ // 128):
                tp = psum_tp.tile([128, 65], F32, tag="tp")
                nc.tensor.transpose(
                    tp[:], accS[:, b * 128:(b + 1) * 128], ident[0:65, 0:65]
                )
                rcp = psml.tile([128, 1], F32, tag="rcp")
                nc.vector.reciprocal(rcp[:], tp[:, D:D + 1])
                nc.vector.tensor_scalar_mul(obuf[:, b, :], tp[:, 0:D], rcp[:])
            nc.sync.dma_start(
                out=out_ap[bh, h * HW_:(h + 1) * HW_, :].rearrange(
                    "(b p) d -> p b d", p=128
                ),
                in_=obuf[:],
            )

    for p in reversed(pools):
        p.release()


_CACHE = {}


def _get_compiled(n_bh=BH):
    key = ("nc", n_bh)
    if key in _CACHE:
        return _CACHE[key]
    nc = bacc.Bacc("TRN2", target_bir_lowering=False, debug=False)
    q = nc.dram_tensor("q", [n_bh, S, D], F32, kind="ExternalInput").ap()
    k = nc.dram_tensor("k", [n_bh, S, D], F32, kind="ExternalInput").ap()
    v = nc.dram_tensor("v", [n_bh, S, D], F32, kind="ExternalInput").ap()
    out = nc.dram_tensor("out", [n_bh, S, D], F32, kind="ExternalOutput").ap()
    with tile.TileContext(nc) as tc:
        build_attention(tc, out, q, k, v, n_bh=n_bh)
    nc.compile()
    _CACHE[key] = nc
    return nc


def kernel(q, k, v):
    nc = _get_compiled()
    qf = np.ascontiguousarray(np.asarray(q), dtype=np.float32).reshape(B * H, S, D)
    kf = np.ascontiguousarray(np.asarray(k), dtype=np.float32).reshape(B * H, S, D)
    vf = np.ascontiguousarray(np.asarray(v), dtype=np.float32).reshape(B * H, S, D)
    in_maps = [
        {
            "q": qf[i * BH:(i + 1) * BH],
            "k": kf[i * BH:(i + 1) * BH],
            "v": vf[i * BH:(i + 1) * BH],
        }
        for i in range(NCORES)
    ]
    res = run_bass_kernel_spmd(nc, in_maps, list(range(NCORES)))
    outs = np.concatenate([res.results[i]["out"] for i in range(NCORES)], axis=0)
    return outs.reshape(B, H, S, D).astype(np.float32)


# revision 10
# speedup vs baseline: 16.6958x; 1.4523x over previous
"""Multi-head attention kernel for Trainium2, sharded over 8 NeuronCores.

Problem: q,k,v [4, 16, 2048, 64] f32 -> softmax(q@k^T/sqrt(64))@v.
Sharding: batch*heads = 64 (b,h) pairs -> 8 per core (no communication).

Per-core algorithm, per (b,h) pair (S=2048, D=64):
  load:   q,k arrive as [128, 16, 64] f32 tiles; each [128, 64] tile is
          PE-transposed (fp32 transpose mode) to [64, 128] in PSUM and
          copied (with f32->bf16 cast) to SBUF by VectorE:
            qT [128, 2048]: d on partitions, duplicated to partitions 64-127
            kT [128, 8*128]: even ki-chunks on partitions 0-63, odd on 64-127
          v is DMA'd by the gpsimd SWDGE with f32->bf16 cast in flight into
          V_aug [128, 16, 128]: cols 0-63 = v, col 64 = ones (softmax
          denominator trick), cols 65-127 = zero padding (enables FWL).
  main:   for each qi-half (1024) x block b (512) x chunk-pair m (2x128 ki):
            S^T pair = kT_pair.T @ qT  (two row-packed bf16 matmuls, K=64,
                                        PE row groups 0-63 / 64-127)
            P^T = exp(S^T / 8)         (one ScalarE ACTIVATE, FD=1024,
                                        PSUM->SBUF, bf16 out)
            acc[:, b] += V_aug^T @ P^T (bf16 matmuls accumulating in PSUM;
                                        row 64 of acc = denominators)
  final:  PE-transpose acc [65, 128]-blocks back -> [128, 65]; multiply by
          VectorE reciprocal of the denominator column; batch the 8 result
          blocks per half into one [128, 8, 64] tile; single DMA to DRAM.

The PE instruction stream is software-pipelined: PV matmuls run SKEW
iterations behind their QK/exp producers so every PE instruction has its
semaphores satisfied before it reaches the head of the queue (PE executes
in order; an unsatisfied wait stalls the whole engine and exposes the
~170ns SBUF latency + 128-cycle drain on every matmul). Next-(b,h) loads/
transposes and previous-half finalize ops drain from a deferred queue at a
bounded rate per iteration, which keeps ScalarE's exp stream (the true
bottleneck, ~275us/core) fed continuously instead of stalling ~8us at
every (b,h boundary.

No max-subtraction is needed: scores ~ N(0,1) after the 1/8 scale, so exp
is far from overflow and softmax is algebraically identical to the
reference.
"""

import numpy as np

import concourse.bass as bass
import concourse.tile as tile
from concourse import bacc, mybir
from concourse.bass_utils import run_bass_kernel_spmd

B, H, S, D = 4, 16, 2048, 64
NCORES = 8
BH = (B * H) // NCORES  # (b,h) pairs per core = 8

F32 = mybir.dt.float32
BF16 = mybir.dt.bfloat16

KC = S // 128    # ki chunks of 128 rows       = 16
NH = 2           # qi halves                    (1024 each)
HW_ = S // NH    # qi-half width                = 1024
NB = HW_ // 512  # 512-wide blocks per half     = 2
NM = KC // 2     # chunk pairs                  = 8
IPB = NH * NB * NM  # iterations per bh pair    = 32
SKEW = 2         # PV runs this many iterations behind QK/exp
DRAIN_RATE = 2   # deferred ops emitted per iteration


def build_attention(tc, out_ap, q_ap, k_ap, v_ap, n_bh=BH):
    nc = tc.nc
    pools = []

    def pool(name, bufs, space="SBUF"):
        p = tc.alloc_tile_pool(name=name, bufs=bufs, space=space)
        pools.append(p)
        return p

    singles = pool("singles", 1)
    pin = pool("pin", 2)        # q/k f32 natural tiles
    pqt = pool("pqt", 2)        # qT / kT bf16
    pv = pool("pv", 2)          # V_aug
    ppt = pool("ppt", 4)        # exp output P^T bf16
    pfin = pool("pfin", 2)      # finalize sbuf staging
    psml = pool("psml", 3)      # small finalize tiles
    pob = pool("pob", 2)        # batched output tiles
    psum_stage = pool("stage", 2, space="PSUM")  # S^T staging, 2 banks each
    psum_acc = pool("acc", 1, space="PSUM")      # PV accumulator, 2 banks
    psum_tp = pool("tp", 2, space="PSUM")        # transposes, 1 bank each

    from concourse.masks import make_identity
    ident = singles.tile([128, 128], F32)
    make_identity(nc, ident[:])

    # deferred ops (loads/transposes/finalize) drained into the main loop
    pending = []

    def drain(n):
        for _ in range(n):
            if pending:
                pending.pop(0)()

    state = {}  # per-bh tiles: qT, kT, vaug

    def push_prefetch(bh):
        """Queue load + transpose + cast ops that produce qT/kT/vaug[bh]."""
        tiles = {}
        state[bh] = tiles

        def dma_qk():
            qstage = pin.tile([128, KC, D], F32, tag="qstage")
            nc.sync.dma_start(
                out=qstage[:],
                in_=q_ap[bh].rearrange("(n p) d -> p n d", p=128),
            )
            kstage = pin.tile([128, KC, D], F32, tag="kstage")
            nc.sync.dma_start(
                out=kstage[:],
                in_=k_ap[bh].rearrange("(n p) d -> p n d", p=128),
            )
            tiles["qstage"], tiles["kstage"] = qstage, kstage
            tiles["qT"] = pqt.tile([128, S], BF16, tag="qT", name="qT")
            tiles["kT"] = pqt.tile(
                [128, NM * 128], BF16, tag="kT", name="kT"
            )

        def dma_v():
            vaug = pv.tile([128, KC, 128], BF16, tag="vaug")
            nc.gpsimd.memset(vaug[:], 0.0)
            nc.gpsimd.memset(vaug[:, :, D:D + 1], 1.0)
            nc.gpsimd.dma_start(
                out=vaug[:, :, 0:D],
                in_=v_ap[bh].rearrange("(n p) d -> p n d", p=128),
            )
            tiles["vaug"] = vaug

        def tq(j):
            def op():
                tp = psum_tp.tile([64, 128], F32, tag="tp")
                nc.tensor.transpose(tp[:], tiles["qstage"][:, j, :], ident[:])
                nc.vector.tensor_copy(
                    tiles["qT"][0:64, j * 128:(j + 1) * 128], tp[:]
                )
            return op

        def tk(j):
            def op():
                tp = psum_tp.tile([64, 128], F32, tag="tp")
                nc.tensor.transpose(tp[:], tiles["kstage"][:, j, :], ident[:])
                row = 64 * (j % 2)
                m = j // 2
                nc.vector.tensor_copy(
                    tiles["kT"][row:row + 64, m * 128:(m + 1) * 128], tp[:]
                )
            return op

        def dup_q():
            nc.vector.tensor_copy(
                tiles["qT"][64:128, :], tiles["qT"][0:64, :]
            )

        pending.append(dma_qk)
        pending.append(dma_v)
        for j in range(KC):
            pending.append(tq(j))
            pending.append(tk(j))
        pending.append(dup_q)

    def push_finalize(bh, h, acc):
        """Queue finalize ops for half h of pair bh (acc in PSUM)."""
        ctx = {}

        def copy_acc():
            accS = pfin.tile([65, HW_], F32, tag="accS")
            nc.vector.tensor_copy(accS[:], acc[0:65, :])
            ctx["accS"] = accS
            ctx["obuf"] = pob.tile(
                [128, HW_ // 128, D], F32, tag="obuf", name="obuf"
            )

        def fin_block(b):
            def op():
                tp = psum_tp.tile([128, 65], F32, tag="tp")
                nc.tensor.transpose(
                    tp[:], ctx["accS"][:, b * 128:(b + 1) * 128],
                    ident[0:65, 0:65],
                )
                rcp = psml.tile([128, 1], F32, tag="rcp")
                nc.vector.reciprocal(rcp[:], tp[:, D:D + 1])
                nc.vector.tensor_scalar_mul(
                    ctx["obuf"][:, b, :], tp[:, 0:D], rcp[:]
                )
            return op

        def store():
            nc.sync.dma_start(
                out=out_ap[bh, h * HW_:(h + 1) * HW_, :].rearrange(
                    "(b p) d -> p b d", p=128
                ),
                in_=ctx["obuf"][:],
            )

        # finalize goes to the FRONT of the queue: the acc PSUM slot must be
        # released promptly (next half's PV matmuls wait on it), and it must
        # not queue behind ~30 next-bh prefetch ops.
        ops = [copy_acc] + [fin_block(b) for b in range(HW_ // 128)] + [store]
        pending[0:0] = ops

    # ---- main software-pipelined loop ----
    push_prefetch(0)
    drain(len(pending))  # first bh loads run up front

    pv_q = []  # deferred PV closures

    for bh in range(n_bh):
        tiles = state[bh]
        if bh + 1 < n_bh:
            push_prefetch(bh + 1)
        acc = None
        for it in range(IPB):
            h, rem = divmod(it, NB * NM)
            b, m = divmod(rem, NM)
            if m == 0 and b == 0:
                acc = psum_acc.tile([128, HW_], F32, tag="acc")
            q0 = h * HW_ + b * 512
            # QK^T row-packed pair -> S^T chunks (2m, 2m+1) x block b
            stage = psum_stage.tile([128, 2, 512], F32, tag="stage")
            nc.tensor.matmul(
                stage[:, 0, :],
                lhsT=tiles["kT"][0:64, m * 128:(m + 1) * 128],
                rhs=tiles["qT"][0:64, q0:q0 + 512],
                start=True, stop=True,
            )
            nc.tensor.matmul(
                stage[:, 1, :],
                lhsT=tiles["kT"][64:128, m * 128:(m + 1) * 128],
                rhs=tiles["qT"][64:128, q0:q0 + 512],
                start=True, stop=True,
            )
            # exp on ScalarE
            pt = ppt.tile([128, 2, 512], BF16, tag="pt")
            nc.scalar.activation(
                pt[:], stage[:], mybir.ActivationFunctionType.Exp, scale=0.125
            )

            # deferred PV for this iteration
            def make_pv(acc_, pt_, vaug_, b_, m_, bh_, h_):
                def op():
                    nc.tensor.matmul(
                        acc_[:, b_ * 512:(b_ + 1) * 512],
                        lhsT=vaug_[:, 2 * m_, :],
                        rhs=pt_[:, 0, :],
                        start=(m_ == 0), stop=False,
                    )
                    nc.tensor.matmul(
                        acc_[:, b_ * 512:(b_ + 1) * 512],
                        lhsT=vaug_[:, 2 * m_ + 1, :],
                        rhs=pt_[:, 1, :],
                        start=False, stop=(m_ == NM - 1),
                    )
                    if m_ == NM - 1 and b_ == NB - 1:
                        push_finalize(bh_, h_, acc_)
                return op

            pv_q.append(make_pv(acc, pt, tiles["vaug"], b, m, bh, h))
            if len(pv_q) > SKEW:
                pv_q.pop(0)()
            drain(DRAIN_RATE)

    while pv_q:
        pv_q.pop(0)()
    while pending:
        drain(1)

    for p in reversed(pools):
        p.release()


_CACHE = {}


def _get_compiled(n_bh=BH):
    key = ("nc", n_bh)
    if key in _CACHE:
        return _CACHE[key]
    nc = bacc.Bacc("TRN2", target_bir_lowering=False, debug=False)
    q = nc.dram_tensor("q", [n_bh, S, D], F32, kind="ExternalInput").ap()
    k = nc.dram_tensor("k", [n_bh, S, D], F32, kind="ExternalInput").ap()
    v = nc.dram_tensor("v", [n_bh, S, D], F32, kind="ExternalInput").ap()
    out = nc.dram_tensor("out", [n_bh, S, D], F32, kind="ExternalOutput").ap()
    with tile.TileContext(nc) as tc:
        build_attention(tc, out, q, k, v, n_bh=n_bh)
    nc.compile()
    _CACHE[key] = nc
    return nc


def kernel(q, k, v):
    nc = _get_compiled()
    qf = np.ascontiguousarray(np.asarray(q), dtype=np.float32).reshape(B * H, S, D)
    kf = np.ascontiguousarray(np.asarray(k), dtype=np.float32).reshape(B * H, S, D)
    vf = np.ascontiguousarray(np.asarray(v), dtype=np.float32).reshape(B * H, S, D)
    in_maps = [
        {
            "q": qf[i * BH:(i + 1) * BH],
            "k": kf[i * BH:(i + 1) * BH],
            "v": vf[i * BH:(i + 1) * BH],
        }
        for i in range(NCORES)
    ]
    res = run_bass_kernel_spmd(nc, in_maps, list(range(NCORES)))
    outs = np.concatenate([res.results[i]["out"] for i in range(NCORES)], axis=0)
    return outs.reshape(B, H, S, D).astype(np.float32)


# revision 11
# speedup vs baseline: 17.8449x; 1.0688x over previous
"""Multi-head attention kernel for Trainium2, sharded over 8 NeuronCores.

Problem: q,k,v [4, 16, 2048, 64] f32 -> softmax(q@k^T/sqrt(64))@v.
Sharding: batch*heads = 64 (b,h) pairs -> 8 per core (no communication).

Per-core algorithm, per (b,h) pair (S=2048, D=64):
  load:   q,k arrive as [128, 16, 64] f32 tiles, are cast to bf16 by
          VectorE, then PE-transposed (bf16, pairs batched per PSUM tile)
          and copied to SBUF:
            qT [128, 2048]: d on partitions, duplicated to partitions 64-127
            kT [128, 8*128]: even ki-chunks on partitions 0-63, odd on 64-127
          (bf16 transposes keep the PE stream free of fp32 matmuls, which
          would disable FWL for the following bf16 weight loads)
          v is DMA'd by the gpsimd SWDGE with f32->bf16 cast in flight into
          V_aug [128, 16, 128]: cols 0-63 = v, col 64 = ones (softmax
          denominator trick), cols 65-127 = zero padding (keeps FWL legal).
  main:   for each qi-half (1024) x block b (512) x chunk-pair m (2x128 ki):
            S^T pair = kT_pair.T @ qT  (two row-packed bf16 matmuls, K=64,
                                        PE row groups 0-63 / 64-127, running
                                        concurrently)
            P^T = exp(S^T / 8)         (one ScalarE ACTIVATE, FD=1024,
                                        PSUM->SBUF, bf16 out)
            acc[:, b] += V_aug^T @ P^T (bf16 matmuls accumulating in PSUM;
                                        row 64 of acc = denominators)
  final:  PE-transpose acc (f32, 4 blocks batched per 1-bank PSUM tile)
          back to [128, 4, 65]; one VectorE reciprocal of the denominator
          column per batch + per-block multiply; batch the 8 result blocks
          per half into one [128, 8, 64] tile; single DMA to DRAM.

The PE instruction stream is software-pipelined: PV matmuls run SKEW
iterations behind their QK/exp producers so every PE instruction has its
semaphores satisfied before it reaches the head of the queue (PE executes
in order; an unsatisfied wait stalls the whole engine and exposes the
~170ns SBUF latency + 128-cycle drain on every matmul). Next-(b,h) loads/
transposes and previous-half finalize ops drain from a deferred queue at a
bounded rate per iteration, keeping ScalarE's exp stream (the bottleneck,
~275us/core) fed continuously.

No max-subtraction is needed: scores ~ N(0,1) after the 1/8 scale, so exp
is far from overflow and softmax is algebraically identical to the
reference.
"""

import numpy as np

import concourse.bass as bass
import concourse.tile as tile
from concourse import bacc, mybir
from concourse.bass_utils import run_bass_kernel_spmd

B, H, S, D = 4, 16, 2048, 64
NCORES = 8
BH = (B * H) // NCORES  # (b,h) pairs per core = 8

F32 = mybir.dt.float32
BF16 = mybir.dt.bfloat16

KC = S // 128    # ki chunks of 128 rows       = 16
NH = 2           # qi halves                    (1024 each)
HW_ = S // NH    # qi-half width                = 1024
NB = HW_ // 512  # 512-wide blocks per half     = 2
NM = KC // 2     # chunk pairs                  = 8
IPB = NH * NB * NM  # iterations per bh pair    = 32
SKEW = 2         # PV runs this many iterations behind QK/exp
DRAIN_RATE = 1   # deferred ops emitted per iteration


def build_attention(tc, out_ap, q_ap, k_ap, v_ap, n_bh=BH):
    nc = tc.nc
    pools = []

    def pool(name, bufs, space="SBUF"):
        p = tc.alloc_tile_pool(name=name, bufs=bufs, space=space)
        pools.append(p)
        return p

    singles = pool("singles", 1)
    pin = pool("pin", 2)        # q/k f32 natural tiles
    pin16 = pool("pin16", 2)    # q/k bf16 natural tiles
    pqt = pool("pqt", 2)        # qT / kT bf16
    pv = pool("pv", 2)          # V_aug
    ppt = pool("ppt", 4)        # exp output P^T bf16
    pfin = pool("pfin", 2)      # finalize sbuf staging
    psml = pool("psml", 3)      # small finalize tiles
    pob = pool("pob", 2)        # batched output tiles
    psum_stage = pool("stage", 2, space="PSUM")  # S^T staging, 2 banks each
    psum_acc = pool("acc", 1, space="PSUM")      # PV accumulator, 2 banks
    psum_tp = pool("tp", 2, space="PSUM")        # transposes, 1 bank each

    from concourse.masks import make_identity
    ident16 = singles.tile([128, 128], BF16)
    make_identity(nc, ident16[:])
    ident = singles.tile([65, 65], F32)
    make_identity(nc, ident[:])

    # deferred ops (loads/transposes/finalize) drained into the main loop
    pending = []

    def drain(n):
        for _ in range(n):
            if pending:
                pending.pop(0)()

    state = {}  # per-bh tiles: qT, kT, vaug

    def push_prefetch(bh):
        """Queue load + cast + transpose ops that produce qT/kT/vaug[bh]."""
        tiles = {}
        state[bh] = tiles

        def dma_qk():
            qstage = pin.tile([128, KC, D], F32, tag="qstage")
            nc.sync.dma_start(
                out=qstage[:],
                in_=q_ap[bh].rearrange("(n p) d -> p n d", p=128),
            )
            kstage = pin.tile([128, KC, D], F32, tag="kstage")
            nc.sync.dma_start(
                out=kstage[:],
                in_=k_ap[bh].rearrange("(n p) d -> p n d", p=128),
            )
            tiles["qstage"], tiles["kstage"] = qstage, kstage
            tiles["qT"] = pqt.tile([128, S], BF16, tag="qT", name="qT")
            tiles["kT"] = pqt.tile(
                [128, NM * 128], BF16, tag="kT", name="kT"
            )

        def dma_v():
            vaug = pv.tile([128, KC, 128], BF16, tag="vaug")
            nc.gpsimd.memset(vaug[:], 0.0)
            nc.gpsimd.memset(vaug[:, :, D:D + 1], 1.0)
            nc.gpsimd.dma_start(
                out=vaug[:, :, 0:D],
                in_=v_ap[bh].rearrange("(n p) d -> p n d", p=128),
            )
            tiles["vaug"] = vaug

        def cast_q():
            q16 = pin16.tile([128, KC, D], BF16, tag="q16", name="q16")
            nc.vector.tensor_copy(q16[:], tiles["qstage"][:])
            tiles["q16"] = q16

        def cast_k():
            k16 = pin16.tile([128, KC, D], BF16, tag="k16", name="k16")
            nc.vector.tensor_copy(k16[:], tiles["kstage"][:])
            tiles["k16"] = k16

        def tq(m):
            # transpose q chunks (2m, 2m+1); contiguous dest -> one copy
            def op():
                tp = psum_tp.tile([64, 2, 128], BF16, tag="tp", name="tpq")
                for i in range(2):
                    nc.tensor.transpose(
                        tp[:, i, :], tiles["q16"][:, 2 * m + i, :], ident16[:]
                    )
                nc.vector.tensor_copy(
                    tiles["qT"][0:64, m * 256:(m + 1) * 256], tp[:]
                )
            return op

        def tk(m):
            # transpose k chunks (2m, 2m+1) -> partition rows 0-63 / 64-127
            def op():
                tp = psum_tp.tile([64, 2, 128], BF16, tag="tp", name="tpk")
                for i in range(2):
                    nc.tensor.transpose(
                        tp[:, i, :], tiles["k16"][:, 2 * m + i, :], ident16[:]
                    )
                nc.vector.tensor_copy(
                    tiles["kT"][0:64, m * 128:(m + 1) * 128], tp[:, 0, :]
                )
                nc.vector.tensor_copy(
                    tiles["kT"][64:128, m * 128:(m + 1) * 128], tp[:, 1, :]
                )
            return op

        def dup_q():
            nc.vector.tensor_copy(
                tiles["qT"][64:128, :], tiles["qT"][0:64, :]
            )

        pending.append(dma_qk)
        pending.append(dma_v)
        pending.append(cast_q)
        pending.append(cast_k)
        for m in range(NM):
            pending.append(tq(m))
            pending.append(tk(m))
        pending.append(dup_q)

    def push_finalize(bh, h, acc):
        """Queue finalize ops for half h of pair bh (acc in PSUM)."""
        ctx = {}

        def copy_acc():
            accS = pfin.tile([65, HW_], F32, tag="accS")
            nc.vector.tensor_copy(accS[:], acc[0:65, :])
            ctx["accS"] = accS
            ctx["obuf"] = pob.tile(
                [128, HW_ // 128, D], F32, tag="obuf", name="obuf"
            )

        def fin_batch(g):
            # transpose 4 blocks into one 1-bank PSUM tile, then divide
            def op():
                tp = psum_tp.tile([128, 4, 65], F32, tag="tp", name="tpf")
                for i in range(4):
                    b = 4 * g + i
                    nc.tensor.transpose(
                        tp[:, i, :], ctx["accS"][:, b * 128:(b + 1) * 128],
                        ident[:],
                    )
                rcp = psml.tile([128, 4], F32, tag="rcp")
                nc.vector.reciprocal(rcp[:], tp[:, :, D])
                for i in range(4):
                    nc.vector.tensor_scalar_mul(
                        ctx["obuf"][:, 4 * g + i, :], tp[:, i, 0:D],
                        rcp[:, i:i + 1],
                    )
            return op

        def store():
            nc.sync.dma_start(
                out=out_ap[bh, h * HW_:(h + 1) * HW_, :].rearrange(
                    "(b p) d -> p b d", p=128
                ),
                in_=ctx["obuf"][:],
            )

        # finalize goes to the FRONT of the queue: the acc PSUM slot must be
        # released promptly (next half's PV matmuls wait on it), and it must
        # not queue behind ~20 next-bh prefetch ops.
        ops = [copy_acc, fin_batch(0), fin_batch(1), store]
        pending[0:0] = ops

    # ---- main software-pipelined loop ----
    push_prefetch(0)
    drain(len(pending))  # first bh loads run up front

    pv_q = []  # deferred PV closures

    for bh in range(n_bh):
        tiles = state[bh]
        if bh + 1 < n_bh:
            push_prefetch(bh + 1)
        acc = None
        for it in range(IPB):
            h, rem = divmod(it, NB * NM)
            b, m = divmod(rem, NM)
            if m == 0 and b == 0:
                acc = psum_acc.tile([128, HW_], F32, tag="acc")
            q0 = h * HW_ + b * 512
            # QK^T row-packed pair -> S^T chunks (2m, 2m+1) x block b
            stage = psum_stage.tile([128, 2, 512], F32, tag="stage")
            nc.tensor.matmul(
                stage[:, 0, :],
                lhsT=tiles["kT"][0:64, m * 128:(m + 1) * 128],
                rhs=tiles["qT"][0:64, q0:q0 + 512],
                start=True, stop=True,
            )
            nc.tensor.matmul(
                stage[:, 1, :],
                lhsT=tiles["kT"][64:128, m * 128:(m + 1) * 128],
                rhs=tiles["qT"][64:128, q0:q0 + 512],
                start=True, stop=True,
            )
            # exp on ScalarE
            pt = ppt.tile([128, 2, 512], BF16, tag="pt")
            nc.scalar.activation(
                pt[:], stage[:], mybir.ActivationFunctionType.Exp, scale=0.125
            )

            # deferred PV for this iteration
            def make_pv(acc_, pt_, vaug_, b_, m_, bh_, h_):
                def op():
                    nc.tensor.matmul(
                        acc_[:, b_ * 512:(b_ + 1) * 512],
                        lhsT=vaug_[:, 2 * m_, :],
                        rhs=pt_[:, 0, :],
                        start=(m_ == 0), stop=False,
                    )
                    nc.tensor.matmul(
                        acc_[:, b_ * 512:(b_ + 1) * 512],
                        lhsT=vaug_[:, 2 * m_ + 1, :],
                        rhs=pt_[:, 1, :],
                        start=False, stop=(m_ == NM - 1),
                    )
                    if m_ == NM - 1 and b_ == NB - 1:
                        push_finalize(bh_, h_, acc_)
                return op

            pv_q.append(make_pv(acc, pt, tiles["vaug"], b, m, bh, h))
            if len(pv_q) > SKEW:
                pv_q.pop(0)()
            drain(DRAIN_RATE)

    while pv_q:
        pv_q.pop(0)()
    while pending:
        drain(1)

    for p in reversed(pools):
        p.release()


_CACHE = {}


def _get_compiled(n_bh=BH):
    key = ("nc", n_bh)
    if key in _CACHE:
        return _CACHE[key]
    nc = bacc.Bacc("TRN2", target_bir_lowering=False, debug=False)
    q = nc.dram_tensor("q", [n_bh, S, D], F32, kind="ExternalInput").ap()
    k = nc.dram_tensor("k", [n_bh, S, D], F32, kind="ExternalInput").ap()
    v = nc.dram_tensor("v", [n_bh, S, D], F32, kind="ExternalInput").ap()
    out = nc.dram_tensor("out", [n_bh, S, D], F32, kind="ExternalOutput").ap()
    with tile.TileContext(nc) as tc:
        build_attention(tc, out, q, k, v, n_bh=n_bh)
    nc.compile()
    _CACHE[key] = nc
    return nc


def kernel(q, k, v):
    nc = _get_compiled()
    qf = np.ascontiguousarray(np.asarray(q), dtype=np.float32).reshape(B * H, S, D)
    kf = np.ascontiguousarray(np.asarray(k), dtype=np.float32).reshape(B * H, S, D)
    vf = np.ascontiguousarray(np.asarray(v), dtype=np.float32).reshape(B * H, S, D)
    in_maps = [
        {
            "q": qf[i * BH:(i + 1) * BH],
            "k": kf[i * BH:(i + 1) * BH],
            "v": vf[i * BH:(i + 1) * BH],
        }
        for i in range(NCORES)
    ]
    res = run_bass_kernel_spmd(nc, in_maps, list(range(NCORES)))
    outs = np.concatenate([res.results[i]["out"] for i in range(NCORES)], axis=0)
    return outs.reshape(B, H, S, D).astype(np.float32)


# revision 16
# speedup vs baseline: 18.8430x; 1.0559x over previous
"""Multi-head attention kernel for Trainium2, sharded over 8 NeuronCores.

Problem: q,k,v [4, 16, 2048, 64] f32 -> softmax(q@k^T/sqrt(64))@v.
Sharding: batch*heads = 64 (b,h) pairs -> 8 per core (no communication).

Per-core algorithm, per (b,h) pair (S=2048, D=64):
  load:   q,k arrive as [128, 16, 64] f32 tiles, are cast to bf16 by
          VectorE, then PE-transposed (bf16, pairs batched per PSUM tile)
          and copied to SBUF:
            qT [128, 2048]: d on partitions, duplicated to partitions 64-127
            kT [128, 8*128]: even ki-chunks on partitions 0-63, odd on 64-127
          (bf16 transposes keep the PE stream free of fp32 matmuls, which
          would disable FWL for the following bf16 weight loads)
          v is DMA'd by the gpsimd SWDGE with f32->bf16 cast in flight into
          V_aug [128, 16, 128]: cols 0-63 = v, col 64 = ones (softmax
          denominator trick), cols 65-127 = zero padding (keeps FWL legal).
  main:   for each qi-half (1024) x block b (512) x chunk-pair m (2x128 ki):
            S^T pair = kT_pair.T @ qT  (two row-packed bf16 matmuls, K=64,
                                        PE row groups 0-63 / 64-127, running
                                        concurrently)
            P^T = exp(S^T / 8)         (one ScalarE ACTIVATE, FD=1024,
                                        PSUM->SBUF, bf16 out)
            acc[:, b] += V_aug^T @ P^T (bf16 matmuls accumulating in PSUM;
                                        row 64 of acc = denominators)
  final:  PE-transpose acc (f32, 4 blocks batched per 1-bank PSUM tile)
          back to [128, 4, 65]; one VectorE reciprocal of the denominator
          column per batch + per-block multiply; batch the 8 result blocks
          per half into one [128, 8, 64] tile; single DMA to DRAM.

The PE instruction stream is software-pipelined: PV matmuls run SKEW
iterations behind their QK/exp producers so every PE instruction has its
semaphores satisfied before it reaches the head of the queue (PE executes
in order; an unsatisfied wait stalls the whole engine and exposes the
~170ns SBUF latency + 128-cycle drain on every matmul). Next-(b,h) loads/
transposes and previous-half finalize ops drain from a deferred queue at a
bounded rate per iteration, keeping ScalarE's exp stream (the bottleneck,
~275us/core) fed continuously.

No max-subtraction is needed: scores ~ N(0,1) after the 1/8 scale, so exp
is far from overflow and softmax is algebraically identical to the
reference.
"""

import numpy as np

import concourse.bass as bass
import concourse.tile as tile
from concourse import bacc, mybir
from concourse.bass_utils import run_bass_kernel_spmd

B, H, S, D = 4, 16, 2048, 64
NCORES = 8
BH = (B * H) // NCORES  # (b,h) pairs per core = 8

F32 = mybir.dt.float32
BF16 = mybir.dt.bfloat16

KC = S // 128    # ki chunks of 128 rows       = 16
NH = 2           # qi halves                    (1024 each)
HW_ = S // NH    # qi-half width                = 1024
NB = HW_ // 512  # 512-wide blocks per half     = 2
NM = KC // 2     # chunk pairs                  = 8
IPB = NH * NB * NM  # iterations per bh pair    = 32
SKEW = 3         # PV runs this many iterations behind QK/exp
DRAIN_RATE = 1   # deferred ops emitted per iteration


def build_attention(tc, out_ap, q_ap, k_ap, v_ap, n_bh=BH):
    nc = tc.nc
    pools = []

    def pool(name, bufs, space="SBUF"):
        p = tc.alloc_tile_pool(name=name, bufs=bufs, space=space)
        pools.append(p)
        return p

    singles = pool("singles", 1)
    pin = pool("pin", 2)        # q/k f32 natural tiles
    pin16 = pool("pin16", 2)    # q/k bf16 natural tiles
    pqt = pool("pqt", 2)        # qT / kT bf16
    pv = pool("pv", 2)          # V_aug
    ppt = pool("ppt", 5)        # exp output P^T bf16
    pfin = pool("pfin", 2)      # finalize sbuf staging
    psml = pool("psml", 3)      # small finalize tiles
    pob = pool("pob", 2)        # batched output tiles
    psum_stage = pool("stage", 2, space="PSUM")  # S^T staging, 2 banks each
    psum_acc = pool("acc", 1, space="PSUM")      # PV accumulator, 2 banks
    psum_tp = pool("tp", 2, space="PSUM")        # transposes, 1 bank each

    from concourse.masks import make_identity
    ident16 = singles.tile([128, 128], BF16)
    make_identity(nc, ident16[:])
    ident = singles.tile([65, 65], F32)
    make_identity(nc, ident[:])

    # Warm the ScalarE exp table (~2.7us ACT_TABLE_LOAD) during the first
    # DMA wait instead of stalling the first real exp.
    warm = singles.tile([128, 1], F32)
    nc.vector.memset(warm[:], 0.0)
    nc.scalar.activation(
        warm[:], warm[:], mybir.ActivationFunctionType.Exp
    )

    # deferred ops (loads/transposes/finalize) drained into the main loop
    pending = []

    def drain(n):
        for _ in range(n):
            if pending:
                pending.pop(0)()

    state = {}  # per-bh tiles: qT, kT, vaug

    def push_prefetch(bh):
        """Queue load + cast + transpose ops that produce qT/kT/vaug[bh]."""
        tiles = {}
        state[bh] = tiles

        def dma_qk():
            qstage = pin.tile([128, KC, D], F32, tag="qstage")
            nc.sync.dma_start(
                out=qstage[:],
                in_=q_ap[bh].rearrange("(n p) d -> p n d", p=128),
            )
            kstage = pin.tile([128, KC, D], F32, tag="kstage")
            nc.sync.dma_start(
                out=kstage[:],
                in_=k_ap[bh].rearrange("(n p) d -> p n d", p=128),
            )
            tiles["qstage"], tiles["kstage"] = qstage, kstage
            tiles["qT"] = pqt.tile([128, S], BF16, tag="qT", name="qT")
            tiles["kT"] = pqt.tile(
                [128, NM * 128], BF16, tag="kT", name="kT"
            )

        def dma_v():
            vaug = pv.tile([128, KC, 128], BF16, tag="vaug")
            nc.gpsimd.memset(vaug[:], 0.0)
            nc.gpsimd.memset(vaug[:, :, D:D + 1], 1.0)
            nc.gpsimd.dma_start(
                out=vaug[:, :, 0:D],
                in_=v_ap[bh].rearrange("(n p) d -> p n d", p=128),
            )
            tiles["vaug"] = vaug

        def cast_q():
            q16 = pin16.tile([128, KC, D], BF16, tag="q16", name="q16")
            nc.vector.tensor_copy(q16[:], tiles["qstage"][:])
            tiles["q16"] = q16

        def cast_k():
            k16 = pin16.tile([128, KC, D], BF16, tag="k16", name="k16")
            nc.vector.tensor_copy(k16[:], tiles["kstage"][:])
            tiles["k16"] = k16

        def tq(m):
            # transpose q chunks (2m, 2m+1); contiguous dest -> one copy,
            # then duplicate into partitions 64-127 right away so early
            # iterations don't wait on a whole-tensor dup
            def op():
                tp = psum_tp.tile([64, 2, 128], BF16, tag="tp", name="tpq")
                for i in range(2):
                    nc.tensor.transpose(
                        tp[:, i, :], tiles["q16"][:, 2 * m + i, :], ident16[:]
                    )
                nc.vector.tensor_copy(
                    tiles["qT"][0:64, m * 256:(m + 1) * 256], tp[:]
                )
                nc.vector.tensor_copy(
                    tiles["qT"][64:128, m * 256:(m + 1) * 256], tp[:]
                )
            return op

        def tk(m):
            # transpose k chunks (2m, 2m+1) -> partition rows 0-63 / 64-127
            def op():
                tp = psum_tp.tile([64, 2, 128], BF16, tag="tp", name="tpk")
                for i in range(2):
                    nc.tensor.transpose(
                        tp[:, i, :], tiles["k16"][:, 2 * m + i, :], ident16[:]
                    )
                nc.vector.tensor_copy(
                    tiles["kT"][0:64, m * 128:(m + 1) * 128], tp[:, 0, :]
                )
                nc.vector.tensor_copy(
                    tiles["kT"][64:128, m * 128:(m + 1) * 128], tp[:, 1, :]
                )
            return op

        pending.append(dma_qk)
        pending.append(dma_v)
        pending.append(cast_q)
        pending.append(cast_k)
        for m in range(NM):
            pending.append(tq(m))
            pending.append(tk(m))

    def push_finalize(bh, h, acc):
        """Queue finalize ops for half h of pair bh (acc in PSUM)."""
        ctx = {}

        def copy_acc():
            accS = pfin.tile([65, HW_], F32, tag="accS")
            nc.vector.tensor_copy(accS[:], acc[0:65, :])
            ctx["accS"] = accS
            ctx["obuf"] = pob.tile(
                [128, HW_ // 128, D], F32, tag="obuf", name="obuf"
            )

        def fin_batch(g):
            # transpose 4 blocks into one 1-bank PSUM tile, then divide
            def op():
                tp = psum_tp.tile([128, 4, 65], F32, tag="tp", name="tpf")
                for i in range(4):
                    b = 4 * g + i
                    nc.tensor.transpose(
                        tp[:, i, :], ctx["accS"][:, b * 128:(b + 1) * 128],
                        ident[:],
                    )
                rcp = psml.tile([128, 4], F32, tag="rcp")
                nc.vector.reciprocal(rcp[:], tp[:, :, D])
                for i in range(4):
                    nc.vector.tensor_scalar_mul(
                        ctx["obuf"][:, 4 * g + i, :], tp[:, i, 0:D],
                        rcp[:, i:i + 1],
                    )
            return op

        def store():
            nc.sync.dma_start(
                out=out_ap[bh, h * HW_:(h + 1) * HW_, :].rearrange(
                    "(b p) d -> p b d", p=128
                ),
                in_=ctx["obuf"][:],
            )

        # finalize goes to the FRONT of the queue: the acc PSUM slot must be
        # released promptly (next half's PV matmuls wait on it), and it must
        # not queue behind ~20 next-bh prefetch ops.
        ops = [copy_acc, fin_batch(0), fin_batch(1), store]
        pending[0:0] = ops

    # ---- main software-pipelined loop ----
    push_prefetch(0)
    drain(len(pending))  # first bh loads run up front

    pv_q = []  # deferred PV closures

    for bh in range(n_bh):
        tiles = state[bh]
        if bh + 1 < n_bh:
            push_prefetch(bh + 1)
        acc = None
        for it in range(IPB):
            h, rem = divmod(it, NB * NM)
            b, m = divmod(rem, NM)
            if m == 0 and b == 0:
                acc = psum_acc.tile([128, HW_], F32, tag="acc")
            q0 = h * HW_ + b * 512
            # QK^T row-packed pair -> S^T chunks (2m, 2m+1) x block b
            stage = psum_stage.tile([128, 2, 512], F32, tag="stage")
            nc.tensor.matmul(
                stage[:, 0, :],
                lhsT=tiles["kT"][0:64, m * 128:(m + 1) * 128],
                rhs=tiles["qT"][0:64, q0:q0 + 512],
                start=True, stop=True,
            )
            nc.tensor.matmul(
                stage[:, 1, :],
                lhsT=tiles["kT"][64:128, m * 128:(m + 1) * 128],
                rhs=tiles["qT"][64:128, q0:q0 + 512],
                start=True, stop=True,
            )
            # exp on ScalarE
            pt = ppt.tile([128, 2, 512], BF16, tag="pt")
            nc.scalar.activation(
                pt[:], stage[:], mybir.ActivationFunctionType.Exp, scale=0.125
            )

            # deferred PV for this iteration
            def make_pv(acc_, pt_, vaug_, b_, m_, bh_, h_):
                def op():
                    nc.tensor.matmul(
                        acc_[:, b_ * 512:(b_ + 1) * 512],
                        lhsT=vaug_[:, 2 * m_, :],
                        rhs=pt_[:, 0, :],
                        start=(m_ == 0), stop=False,
                    )
                    nc.tensor.matmul(
                        acc_[:, b_ * 512:(b_ + 1) * 512],
                        lhsT=vaug_[:, 2 * m_ + 1, :],
                        rhs=pt_[:, 1, :],
                        start=False, stop=(m_ == NM - 1),
                    )
                    if m_ == NM - 1 and b_ == NB - 1:
                        push_finalize(bh_, h_, acc_)
                return op

            pv_q.append(make_pv(acc, pt, tiles["vaug"], b, m, bh, h))
            if len(pv_q) > SKEW:
                pv_q.pop(0)()
            drain(DRAIN_RATE)

    while pv_q:
        pv_q.pop(0)()
    while pending:
        drain(1)

    for p in reversed(pools):
        p.release()


_CACHE = {}


def _get_compiled(n_bh=BH):
    key = ("nc", n_bh)
    if key in _CACHE:
        return _CACHE[key]
    nc = bacc.Bacc("TRN2", target_bir_lowering=False, debug=False)
    q = nc.dram_tensor("q", [n_bh, S, D], F32, kind="ExternalInput").ap()
    k = nc.dram_tensor("k", [n_bh, S, D], F32, kind="ExternalInput").ap()
    v = nc.dram_tensor("v", [n_bh, S, D], F32, kind="ExternalInput").ap()
    out = nc.dram_tensor("out", [n_bh, S, D], F32, kind="ExternalOutput").ap()
    with tile.TileContext(nc) as tc:
        build_attention(tc, out, q, k, v, n_bh=n_bh)
    nc.compile()
    _CACHE[key] = nc
    return nc


def kernel(q, k, v):
    nc = _get_compiled()
    qf = np.ascontiguousarray(np.asarray(q), dtype=np.float32).reshape(B * H, S, D)
    kf = np.ascontiguousarray(np.asarray(k), dtype=np.float32).reshape(B * H, S, D)
    vf = np.ascontiguousarray(np.asarray(v), dtype=np.float32).reshape(B * H, S, D)
    in_maps = [
        {
            "q": qf[i * BH:(i + 1) * BH],
            "k": kf[i * BH:(i + 1) * BH],
            "v": vf[i * BH:(i + 1) * BH],
        }
        for i in range(NCORES)
    ]
    res = run_bass_kernel_spmd(nc, in_maps, list(range(NCORES)))
    outs = np.concatenate([res.results[i]["out"] for i in range(NCORES)], axis=0)
    return outs.reshape(B, H, S, D).astype(np.float32)
